# revision 45
# baseline (speedup 1.0000x reference)
"""Trainium2 Bass kernel for nn_BoneRefusion (17-group BoneMLP over [B,T,16,3]).

Pure data parallel over batch across 8 cores; ~128.7 us/core measured
(2.33x over the 299-us predecessor). The PE runs at the cold 1.2 GHz
clock in this environment (HAM never opens; verified by a warmup-burst
experiment), so matmul streams cost N/1.2 ns.

Main pass (groups 0-15 = 256 hidden features/token), per 512-pair block
exactly 3 PE stream slots AND both evac engines ~100% - the dual floor:
  - Host packs inputs feature-major, 2-set: sbuf [112, nb], set A's 48
    features at partitions 0-47, set B's at 64-111.
  - L1 = 2 slots (hidden chunks of 128), each = two ROW-TILED matmuls
    (A in PE rows 0-63, B in rows 64-127) streaming concurrently.
  - L2 = 1 slot of 4 COLUMN-TILED matmuls (M=24 at cols 0/32/64/96),
    emitted one block late (software pipelining) so the in-order tensor
    queue never stalls on evacuation latency; tiles in readiness order.
  - Evacuations uniform per block so no queue exceeds the period:
    Vector takes the c0 psum pair (tensor_scalar add+max = b1+ReLU),
    Scalar the c1 pair (activation Relu+bias); the L2 out evacuation
    (+b2) splits 3/8 Vector / 5/8 Scalar (balances their clocks and
    per-instruction overheads). Full-tile h evacs release both banks
    at once, keeping the L1 matmul pairs co-issued.
  - PSUM: one 6-bank pool (bufs=3) for L1 + 2 banks for L2 out.

Tail pass (group 16; limbs [13,3] -> 6 inputs, 16 hidden): 16-set
packed (K = 8 sets x 6 feats per row-tile, M = 8 sets x 16 hidden) so
the ragged group runs at full PE utilization instead of poisoning every
block; input prefetched mid-main-loop; per-pair output DMAs.

DMA: block-contiguous HBM layouts, 4 blocks per transfer (4 KB per
partition row) to amortize HWDGE descriptor generation; group 0 split
so block 0's data lands first; weights/bias ride the Scalar HWDGE
queue in parallel. Output is bf16 (total rel-err ~2.7e-3); host casts
to fp32. All matmuls bf16; psum fp32.
"""

import sys

import numpy as np
import ml_dtypes

sys.path.insert(0, "/opt/trn_rl_repo")

import concourse.bass as bass
import concourse.mybir as mybir
import concourse.tile as tile
from concourse import bacc
from concourse.bass_utils import run_bass_kernel_spmd

BF16 = mybir.dt.bfloat16
F32 = mybir.dt.float32
BF16_NP = ml_dtypes.bfloat16

LIMBS = [[0, 1, 2], [3, 4, 5], [6, 7], [8, 9], [10, 11, 12], [13, 14, 15],
         [6, 7, 1, 2], [6, 7, 4, 5], [6, 7, 11, 12], [6, 7, 14, 15], [6, 7, 9],
         [14, 15, 11, 12], [1, 2, 4, 5], [14, 15, 4, 5], [11, 12, 4, 5],
         [10, 0], [13, 3]]
NG = 17
HID = 16
B, T, NJ, C = 2048, 243, 16, 3
NF = NJ * C                    # 48 input features per token
NCORES = 8
BC = B // NCORES               # batches per core
TC = BC * T                    # tokens per core (62208)
S = TC // 2                    # token pairs per core (31104)
NBLK = 512                     # token-pairs per block (= one psum bank)
NBLOCKS = (S + NBLK - 1) // NBLK          # 61 (60 full + 384)
GRP = 4                        # blocks per DMA group
NGRPS = (NBLOCKS + GRP - 1) // GRP        # 16
KX = 112                       # sbuf input rows: A 0-47, pad, B 64-111

# ---- tail (group 16) geometry ----
G16_FEATS = [13 * C + 0, 13 * C + 1, 13 * C + 2, 3 * C + 0, 3 * C + 1, 3 * C + 2]
NSETS = 16
TT = TC // NSETS               # tokens per set (3888)
TBLK = 512
NTB = (TT + TBLK - 1) // TBLK  # 8 tail batches (7 full + 272)
TT_LAST = TT - (NTB - 1) * TBLK

# stationary-weight sbuf layout (one [128, 536] bf16 tile):
#   cols 0-127   w1 chunk0 (A rows 0-47, B rows 64-111)
#   cols 128-255 w1 chunk1
#   cols 256-383 w2 main (4 col tiles of M=24 at 256+32j)
#   cols 384-511 w1 tail (8-set block diag, dup at rows 64-111)
#   cols 512-535 w2 tail (8-set block diag, K=128)
WST_COLS = 536


def _host_weights(W1, b1, W2, b2, idx):
    W1 = np.asarray(W1, np.float32)
    b1 = np.asarray(b1, np.float32)
    W2 = np.asarray(W2, np.float32)
    b2 = np.asarray(b2, np.float32)
    idx = np.asarray(idx)

    # dense [48, 272] W1; padded limb rows of W1 are zero so += handles dups
    w1full = np.zeros((NF, NG * HID), np.float32)
    for g in range(NG):
        for j in range(4):
            r = int(idx[g, j]) * C
            w1full[r:r + C, g * HID:(g + 1) * HID] += W1[g, j * C:(j + 1) * C, :]
    b1flat = b1.reshape(NG * HID)

    wst = np.zeros((128, WST_COLS), np.float32)
    # w1 main chunks
    for c in range(2):
        blk = w1full[:, 128 * c:128 * (c + 1)]
        wst[0:48, 128 * c:128 * (c + 1)] = blk
        wst[64:112, 128 * c:128 * (c + 1)] = blk
    # w2 main col tiles: j=0 A g0-7, j=1 A g8-15, j=2 B g0-7, j=3 B g8-15
    for j in range(4):
        g0 = 8 * (j % 2)
        col = 256 + 32 * j
        for g in range(8):
            wst[16 * g:16 * g + 16, col + 3 * g:col + 3 * g + 3] = W2[g0 + g]
    # w1 tail: 8-set block diag of [6, 16], dup at rows 64-111
    w1t = W1[16, 0:6, :]                       # [6, 16]
    for s in range(8):
        wst[6 * s:6 * s + 6, 384 + 16 * s:384 + 16 * s + 16] = w1t
        wst[64 + 6 * s:64 + 6 * s + 6, 384 + 16 * s:384 + 16 * s + 16] = w1t
    # w2 tail: 8-set block diag of [16, 3]
    for s in range(8):
        wst[16 * s:16 * s + 16, 512 + 3 * s:512 + 3 * s + 3] = W2[16]

    # biases, per psum partition: [128, 5] f32
    bias = np.zeros((128, 5), np.float32)
    bias[:, 0] = b1flat[0:128]
    bias[:, 1] = b1flat[128:256]
    boa = np.zeros(128, np.float32)
    for half in range(2):                      # 0 = A (parts 0-63), 1 = B
        for j in range(2):                     # chunk (g0-7 / g8-15)
            base = 64 * half + 32 * j
            boa[base:base + 24] = b2[8 * j:8 * j + 8].reshape(-1)
    bias[:, 2] = boa
    bias[:, 3] = np.tile(b1[16], 8)
    b2t = np.zeros(128, np.float32)
    for m in range(4):
        b2t[32 * m:32 * m + 24] = np.tile(b2[16], 8)
    bias[:, 4] = b2t

    return wst.astype(BF16_NP), bias


def _build_nc():
    nc = bacc.Bacc(
        "TRN2", target_bir_lowering=False, debug=False, num_devices=NCORES,
    )
    x2 = nc.dram_tensor("x2", [NGRPS * KX, GRP * NBLK], BF16,
                        kind="ExternalInput").ap()
    x16 = nc.dram_tensor("x16", [KX, NTB * TBLK], BF16,
                         kind="ExternalInput").ap()
    wst = nc.dram_tensor("wst", [128, WST_COLS], BF16, kind="ExternalInput").ap()
    bias = nc.dram_tensor("bias", [128, 5], F32, kind="ExternalInput").ap()
    outm = nc.dram_tensor("outm", [NGRPS * 120, GRP * NBLK], BF16,
                          kind="ExternalOutput").ap()
    outt = nc.dram_tensor("outt", [120, 4 * TBLK], BF16,
                          kind="ExternalOutput").ap()

    with tile.TileContext(nc) as tc:
        with (
            tc.tile_pool(name="singles", bufs=1) as singles,
            tc.tile_pool(name="xin", bufs=3) as xin,
            tc.tile_pool(name="hsb", bufs=3) as hsb,
            tc.tile_pool(name="osb", bufs=2) as osb,
        ):
            # weights/bias go via the Scalar HWDGE queue so they issue in
            # parallel with the first input DMA on Sync.
            wst_sb = singles.tile([128, WST_COLS], BF16)
            nc.gpsimd.dma_start(wst_sb, wst)
            # tail input DMA'd mid-main-loop (prefetch without delaying start)
            x16_sb = singles.tile([KX, NTB * TBLK], BF16)
            bias_sb = singles.tile([128, 5], F32)
            nc.gpsimd.dma_start(bias_sb, bias)
            b1c = [bias_sb[:, 0:1], bias_sb[:, 1:2]]
            boa = bias_sb[0:120, 2:3]
            b1t = bias_sb[:, 3:4]
            b2t = bias_sb[0:120, 4:5]

            # ---------------- main pass: groups 0-15 ----------------
            # Software-pipelined: iteration i emits L1+evac for block i and
            # L2+out-evac for block i-1, so the tensor queue never stalls
            # on the evac latency (in-order engine queues).
            with (
                tc.tile_pool(name="pcx", bufs=3, space="PSUM") as pcx,
                tc.tile_pool(name="poa", bufs=2, space="PSUM") as poa,
            ):
                ots = {}                      # group -> out sbuf tile
                prev = None                   # (ht, b, nb) of block i-1

                def emit_l2(pv):
                    ht_p, b_p, nb_p = pv
                    op = poa.tile([128, NBLK], F32, tag="op")
                    for j in (0, 2, 1, 3):    # operand-readiness order
                        cch, st = j % 2, j // 2
                        nc.tensor.matmul(
                            op[32 * j:32 * j + 24, :nb_p],
                            lhsT=wst_sb[:, 256 + 32 * j:256 + 32 * j + 24],
                            rhs=ht_p[:, cch, st, :nb_p],
                            start=True, stop=True,
                            tile_position=(0, 32 * j),
                        )
                    gp = b_p // GRP
                    otp = ots[gp]
                    cp = (b_p % GRP) * NBLK
                    hf = (3 * nb_p) // 8
                    nc.vector.tensor_scalar(
                        otp[:, cp:cp + hf], op[0:120, :hf], boa, None,
                        mybir.AluOpType.add)
                    nc.scalar.activation(
                        out=otp[:, cp + hf:cp + nb_p], in_=op[0:120, hf:nb_p],
                        func=mybir.ActivationFunctionType.Identity,
                        bias=boa, scale=1.0)
                    if b_p % GRP == GRP - 1 or b_p == NBLOCKS - 1:
                        gc = min(GRP * NBLK, S - gp * GRP * NBLK)
                        nc.sync.dma_start(
                            outm[gp * 120:(gp + 1) * 120, :gc], otp[:, :gc])
                        del ots[gp]

                for b in range(NBLOCKS):
                    if b == NBLOCKS // 2:
                        half = NTB * TBLK // 2
                        nc.sync.dma_start(x16_sb[:, :half], x16[:, :half])
                        nc.sync.dma_start(x16_sb[:, half:], x16[:, half:])
                    sl = b % GRP
                    if sl == 0:
                        g = b // GRP
                        xt = xin.tile([KX, GRP * NBLK], BF16, tag="xt")
                        g_cols = min(GRP * NBLK, S - g * GRP * NBLK)
                        if g == 0:
                            nc.sync.dma_start(xt[:, :NBLK], x2[0:KX, :NBLK])
                            nc.sync.dma_start(xt[:, NBLK:g_cols],
                                              x2[0:KX, NBLK:g_cols])
                        else:
                            nc.sync.dma_start(
                                xt[:, :g_cols],
                                x2[g * KX:(g + 1) * KX, :g_cols])
                        ot = osb.tile([120, GRP * NBLK], BF16, tag="ot")
                        ots[g] = ot
                    nb = min(NBLK, S - b * NBLK)
                    c0 = sl * NBLK
                    # ---- L1: 2 chunk-batches, row-tiled A/B ----
                    hp0 = pcx.tile([128, 2, NBLK], F32, tag="hp")
                    hp1 = pcx.tile([128, 2, NBLK], F32, tag="hp", name="hp1")
                    hp = [hp0, hp1]
                    for cch in range(2):
                        nc.tensor.matmul(
                            hp[cch][:, 0, :nb],
                            lhsT=wst_sb[0:48, 128 * cch:128 * (cch + 1)],
                            rhs=xt[0:48, c0:c0 + nb],
                            start=True, stop=True,
                        )
                        nc.tensor.matmul(
                            hp[cch][:, 1, :nb],
                            lhsT=wst_sb[64:112, 128 * cch:128 * (cch + 1)],
                            rhs=xt[64:112, c0:c0 + nb],
                            start=True, stop=True,
                        )
                    # ---- evac h with bias+relu: full tiles, V gets c0,
                    # S gets c1 (coarse WAR release keeps MM pairs synced) --
                    ht = hsb.tile([128, 2, 2, NBLK], BF16, tag="ht")
                    nc.vector.tensor_scalar(
                        ht[:, 0, :, :nb], hp[0][:, :, :nb], b1c[0], 0.0,
                        mybir.AluOpType.add, mybir.AluOpType.max)
                    nc.scalar.activation(
                        out=ht[:, 1, :, :nb], in_=hp[1][:, :, :nb],
                        func=mybir.ActivationFunctionType.Relu,
                        bias=b1c[1], scale=1.0)
                    # ---- L2 + out-evac for the previous block ----
                    if prev is not None:
                        emit_l2(prev)
                    prev = (ht, b, nb)
                emit_l2(prev)

            # ---------------- tail pass: group 16 ----------------
            with (
                tc.tile_pool(name="pt", bufs=3, space="PSUM") as pt,
                tc.tile_pool(name="pot", bufs=2, space="PSUM") as pot,
                tc.tile_pool(name="h16p", bufs=3) as h16p,
                tc.tile_pool(name="o16p", bufs=1) as o16p,
            ):
                o16 = o16p.tile([120, 4 * TBLK], BF16)
                state = {"opt": None}
                prev_t = None

                def emit_l2_tail(pv):
                    h16_p, k_p, nb_p = pv
                    if k_p % 2 == 0:
                        state["opt"] = pot.tile([128, TBLK], F32, tag="opt",
                                                name="opt_t")
                    opt_t = state["opt"]
                    ofs = 64 * (k_p % 2)
                    nc.tensor.matmul(
                        opt_t[ofs:ofs + 24, :nb_p],
                        lhsT=wst_sb[:, 512:536],
                        rhs=h16_p[:, 0, :nb_p],
                        start=True, stop=True,
                        tile_position=(0, ofs),
                    )
                    nc.tensor.matmul(
                        opt_t[ofs + 32:ofs + 56, :nb_p],
                        lhsT=wst_sb[:, 512:536],
                        rhs=h16_p[:, 1, :nb_p],
                        start=True, stop=True,
                        tile_position=(0, ofs + 32),
                    )
                    if k_p % 2 == 1:
                        q = k_p // 2
                        nc.vector.tensor_scalar(
                            o16[:, q * TBLK:q * TBLK + TBLK // 2],
                            opt_t[0:120, :TBLK // 2], b2t, None,
                            mybir.AluOpType.add)
                        nc.scalar.activation(
                            out=o16[:, q * TBLK + TBLK // 2:(q + 1) * TBLK],
                            in_=opt_t[0:120, TBLK // 2:],
                            func=mybir.ActivationFunctionType.Identity,
                            bias=b2t, scale=1.0)
                        nc.sync.dma_start(
                            outt[:, q * TBLK:(q + 1) * TBLK],
                            o16[:, q * TBLK:(q + 1) * TBLK])

                for k in range(NTB):
                    nb = TBLK if k < NTB - 1 else TT_LAST
                    col = k * TBLK
                    htp = pt.tile([128, 2, TBLK], F32, tag="htp")
                    nc.tensor.matmul(
                        htp[:, 0, :nb],
                        lhsT=wst_sb[0:48, 384:512],
                        rhs=x16_sb[0:48, col:col + nb],
                        start=True, stop=True,
                    )
                    nc.tensor.matmul(
                        htp[:, 1, :nb],
                        lhsT=wst_sb[64:112, 384:512],
                        rhs=x16_sb[64:112, col:col + nb],
                        start=True, stop=True,
                    )
                    h16 = h16p.tile([128, 2, TBLK], BF16, tag="h16")
                    nc.vector.tensor_scalar(
                        h16[:, 0, :nb], htp[:, 0, :nb], b1t, 0.0,
                        mybir.AluOpType.add, mybir.AluOpType.max)
                    nc.scalar.activation(
                        out=h16[:, 1, :nb], in_=htp[:, 1, :nb],
                        func=mybir.ActivationFunctionType.Relu,
                        bias=b1t, scale=1.0)
                    if prev_t is not None:
                        emit_l2_tail(prev_t)
                    prev_t = (h16, k, nb)
                emit_l2_tail(prev_t)   # final pair's DMA fires inside
    nc.finalize()
    return nc


_NC_CACHE = None


def _get_nc():
    global _NC_CACHE
    if _NC_CACHE is None:
        _NC_CACHE = _build_nc()
    return _NC_CACHE


def _pack_core_inputs(xc):
    """xc: [TC, 48] fp32 for one core -> (x2, x16) bf16 arrays."""
    xa = np.ascontiguousarray(xc[:S].T)        # [48, S]
    xb = np.ascontiguousarray(xc[S:].T)
    x2 = np.zeros((NGRPS, KX, GRP * NBLK), BF16_NP)
    full = NGRPS * GRP * NBLK
    pad = full - S
    xa_p = np.pad(xa, ((0, 0), (0, pad)))
    xb_p = np.pad(xb, ((0, 0), (0, pad)))
    x2[:, 0:48, :] = xa_p.reshape(48, NGRPS, GRP * NBLK).transpose(1, 0, 2)
    x2[:, 64:112, :] = xb_p.reshape(48, NGRPS, GRP * NBLK).transpose(1, 0, 2)

    xg = np.ascontiguousarray(xc[:, G16_FEATS])        # [TC, 6]
    xg = xg.reshape(NSETS, TT, 6)
    x16 = np.zeros((KX, NTB * TBLK), BF16_NP)
    padt = NTB * TBLK - TT
    for s in range(8):
        x16[6 * s:6 * s + 6, :TT] = xg[s].T
        x16[64 + 6 * s:64 + 6 * s + 6, :TT] = xg[8 + s].T
    return x2.reshape(NGRPS * KX, GRP * NBLK), x16


def _unpack_core_output(om, ot_):
    """om: [NGRPS*120, GRP*NBLK] bf16; ot_: [120, 4*TBLK] bf16 ->
    oc [TC, 17, 3] f32."""
    oc = np.empty((TC, NG, C), np.float32)
    om = np.asarray(om, np.float32).reshape(NGRPS, 120, GRP * NBLK)
    om = om.transpose(1, 0, 2).reshape(120, NGRPS * GRP * NBLK)[:, :S]
    for half, t0 in ((0, 0), (1, S)):          # A tokens then B tokens
        for j in range(2):                     # chunk -> groups 8j..8j+7
            rows = om[64 * half + 32 * j:64 * half + 32 * j + 24]
            oc[t0:t0 + S, 8 * j:8 * j + 8, :] = (
                rows.T.reshape(S, 8, C))
    ot_ = np.asarray(ot_, np.float32).reshape(120, 4, TBLK)
    for k in range(NTB):
        nb = TBLK if k < NTB - 1 else TT_LAST
        q, ofs = k // 2, 64 * (k % 2)
        blkcols = ot_[:, q, :nb]               # [120, nb]
        for s8 in range(2):                    # T0 (sets 0-7) / T1 (8-15)
            rows = blkcols[ofs + 32 * s8:ofs + 32 * s8 + 24]   # [24, nb]
            sets = np.arange(8) + 8 * s8
            toks = sets[:, None] * TT + k * TBLK + np.arange(nb)[None, :]
            oc[toks.reshape(-1), 16, :] = (
                rows.reshape(8, C, nb).transpose(0, 2, 1).reshape(-1, C))
    return oc


def _kernel_impl(x, W1, b1, W2, b2, idx, _want_trace=False):
    x = np.asarray(x, np.float32)
    wst, bias = _host_weights(W1, b1, W2, b2, idx)

    in_maps = []
    for c in range(NCORES):
        xc = x[c * BC:(c + 1) * BC].reshape(TC, NF)
        x2, x16 = _pack_core_inputs(xc)
        in_maps.append({"x2": x2, "x16": x16, "wst": wst, "bias": bias})

    nc = _get_nc()
    res = run_bass_kernel_spmd(
        nc, in_maps, core_ids=list(range(NCORES)), trace=_want_trace,
    )

    out = np.empty((B, T, NG, C), np.float32)
    for c in range(NCORES):
        oc = _unpack_core_output(res.results[c]["outm"], res.results[c]["outt"])
        out[c * BC:(c + 1) * BC] = oc.reshape(BC, T, NG, C)
    return out, res


def kernel(**inputs):
    out, _ = _kernel_impl(**inputs)
    return out
